# revision 4
# baseline (speedup 1.0000x reference)
"""MoE ragged FFN kernel for Trainium2 (8 NeuronCores, pair-balanced).

v4 = v3 (fused, act SBUF-resident, H halves) + load balancing: experts are
paired (largest count with smallest), each pair's tokens split across two
cores. Every core runs two segments — slot1 tokens of expert A, slot2 of
expert B — so capacity drops from max(count) to
even_ceil(max_big/2) + even_ceil(max_small/2) (~mean), cutting padded
compute on every core. Each core streams both experts' weights (wg/wl get a
leading [2] segment dim).

The traced path stages inputs on device once, runs `warm` untraced launches
back-to-back, then captures the NTFF profile of a final launch — measuring
steady-state (power-controller ramped) rather than cold-start behavior.
"""

import os
import tempfile

import numpy as np

P = 128
RMS_EPS = 1e-6

MOE_DTYPE = os.environ.get("MOE_DTYPE", "f16")
MOE_WARM = int(os.environ.get("MOE_WARM", "1"))

_NEFF_CACHE: dict = {}
_RUNNER_CACHE: dict = {}


def _route_numpy(x, w_router, router_scale, top_k):
    G, S, F = x.shape
    B = G * S
    var = np.mean(np.square(x), axis=-1, keepdims=True, dtype=np.float32)
    ri = x / np.sqrt(var + RMS_EPS)
    ri = ri * np.float32(1.0 / np.sqrt(np.float32(F))) * router_scale
    logits = (ri.reshape(B, F) @ w_router).astype(np.float32)
    m = logits.max(axis=-1, keepdims=True)
    e = np.exp(logits - m)
    probs = e / e.sum(axis=-1, keepdims=True)
    choices = np.argsort(-logits, axis=-1, kind="stable")[:, :top_k]
    sel = np.take_along_axis(probs, choices, axis=-1)
    renorm = sel.sum(axis=-1, keepdims=True)
    renorm = np.where(renorm > 0.0, renorm, np.float32(1.0))
    combine = (sel / renorm).astype(np.float32)
    return choices.astype(np.int64), combine


def _route(x, w_router, router_scale, top_k):
    """Reference-exact router on CPU via jax. Returns (choices, combine)."""
    try:
        import jax
        import jax.numpy as jnp

        cpu = jax.devices("cpu")[0]
    except Exception:
        return _route_numpy(np.asarray(x, dtype=np.float32),
                            np.asarray(w_router), np.asarray(router_scale),
                            top_k)
    G, S, F = x.shape
    E = w_router.shape[1]
    with jax.default_device(cpu):
        xj = jax.device_put(np.asarray(x), cpu)
        wj = jax.device_put(np.asarray(w_router), cpu)
        rj = jax.device_put(np.asarray(router_scale), cpu)
        var = jnp.mean(jnp.square(xj), axis=-1, keepdims=True)
        ri = xj * jax.lax.rsqrt(var + RMS_EPS)
        root_size = jax.lax.rsqrt(jnp.array(F, dtype=ri.dtype))
        ri = ri * root_size * rj.astype(ri.dtype)
        logits = jnp.einsum("gsd,de->gse", ri, wj).astype(jnp.float32)
        probs = jax.nn.softmax(logits, axis=-1)
        _, choices = jax.lax.approx_max_k(logits, k=top_k)
        indicator = jax.nn.one_hot(choices, E, dtype=probs.dtype).sum(axis=-2)
        renorm = jnp.sum(indicator * probs, axis=-1, keepdims=True)
        renorm = jnp.where(renorm > 0.0, renorm, 1.0)
        weights = probs / renorm
        combine = jnp.take_along_axis(weights, choices, axis=-1)
    B = G * S
    return (
        np.asarray(choices).reshape(B, top_k),
        np.asarray(combine).reshape(B, top_k).astype(np.float32),
    )


def _mm_dt(mybir, dtype_name):
    return {
        "f32r": mybir.dt.float32r,
        "bf16": mybir.dt.bfloat16,
        "f16": mybir.dt.float16,
    }[dtype_name]


def _np_in_dtype():
    if MOE_DTYPE == "f32r":
        return np.float32
    if MOE_DTYPE == "f16":
        return np.float16
    import ml_dtypes

    return ml_dtypes.bfloat16


def _chunk_sizes(C, cap=512):
    """Near-equal even chunk sizes <= cap summing to C."""
    assert C % 2 == 0
    nchunk = -(-C // cap)
    base = 2 * (C // (2 * nchunk))
    rem = (C - nchunk * base) // 2
    return [base + 2 * (c < rem) for c in range(nchunk)]


def _plan_segments(counts, idx_per_e, E):
    """Choose uniform per-core slot sizes and the expert->slot-instance
    assignment. Tries a 3-segment tercile template (top-2 experts spread over
    3 cores each), falls back to big/small pairing, then to one expert/core.

    Returns (slots, core_segs) where core_segs[core] is a list of
    (expert, token_copy_indices) per segment."""
    def ec(v, n):                     # even ceil of v/n
        return max(2, 2 * (-(-int(v) // (2 * n))))

    order = [int(e) for e in np.argsort(-counts, kind="stable")]
    c = [int(counts[e]) for e in order]

    plans = []
    if E == 8:
        # --- 3-seg template ---
        s1 = max(ec(c[0], 3), ec(c[1], 3))
        s2 = max(ec(max(c[2] - s1, 0), 2), ec(max(c[3] - s1, 0), 2))
        s3 = max(ec(max(c[i] - s2, 0), 2) for i in range(4, 8))
        # slot instances per core, in slot order; entries are expert-rank
        a1 = [0, 0, 0, 1, 1, 1, 2, 3]
        a2 = [2, 2, 3, 3, 4, 5, 6, 7]
        a3 = [4, 4, 5, 5, 6, 6, 7, 7]
        if min(s1, s2, s3) >= 300:
            plans.append(([s1, s2, s3], [a1, a2, a3]))
        # --- 2-seg pairing ---
        p1 = max(ec(c[i], 2) for i in range(4))
        p2 = max(ec(c[i], 2) for i in range(4, 8))
        a1 = [0, 0, 1, 1, 2, 2, 3, 3]
        a2 = [7, 7, 6, 6, 5, 5, 4, 4]
        if min(p1, p2) >= 300:
            plans.append(([p1, p2], [a1, a2]))
    # --- 1-seg fallback: one expert per core ---
    if E == 8:
        s = max(512, 2 * (-(-int(counts.max()) // 2)))
        plans.append(([s], [list(range(8))]))

    slots, assign = min(plans, key=lambda p: sum(p[0]))

    # Split each expert's token list sequentially over its slot instances
    # (core-major order within each slot row, slot rows in order).
    core_segs = [[] for _ in range(E)]
    taken = {e: 0 for e in range(E)}
    for s, row in enumerate(assign):
        for core in range(E):
            e = order[row[core]]
            lo = taken[e]
            part = idx_per_e[e][lo:lo + slots[s]]
            taken[e] = lo + len(part)
            core_segs[core].append((e, part))
    for e in range(E):
        assert taken[e] == len(idx_per_e[e]), (
            f"segment plan dropped tokens for expert {e}")
    return slots, core_segs


def _seg_chunks(slots):
    """Per-segment chunk lists. The 432 cap keeps chunks near-equal ACROSS
    segments too, so the x tile (padded to the global max chunk) stays small
    enough for SBUF."""
    sizes, seg_of = [], []
    for s, slot in enumerate(slots):
        cs = _chunk_sizes(slot, cap=432)
        sizes += cs
        seg_of += [s] * len(cs)
    return sizes, seg_of


def _build_nc_fused(slots, F, H, dtype_name):
    import concourse.mybir as mybir
    import concourse.tile as tile
    from concourse import bacc

    KF = F // P          # stage-1 contraction subtiles
    KH = H // P
    KH2 = KH // 2        # stage-2 contraction subtiles per half
    MG = 2 * H // P      # wg column tiles, gate/lin interleaved per 128
    MO = F // P          # output row tiles
    C = sum(slots)
    NSEG = len(slots)
    sizes, seg_of = _seg_chunks(slots)
    offs = np.concatenate([[0], np.cumsum(sizes)]).tolist()
    nchunk = len(sizes)
    TBmax = max(sizes)
    f32 = mybir.dt.float32
    dt_in = _mm_dt(mybir, dtype_name)

    nc = bacc.Bacc(None, target_bir_lowering=False)
    xc = nc.dram_tensor("xc", [nchunk, P, KF, TBmax], dt_in,
                        kind="ExternalInput")
    wg = nc.dram_tensor("wg", [NSEG, P, MG * KF, P], dt_in,
                        kind="ExternalInput")
    wl = nc.dram_tensor("wl", [NSEG, P, MO, KH, P], dt_in,
                        kind="ExternalInput")
    yT = nc.dram_tensor("yT", [2, MO, P, C], f32, kind="ExternalOutput")

    with tile.TileContext(nc) as tc:
        with (
            tc.tile_pool(name="xp", bufs=1) as xp,
            tc.tile_pool(name="actp", bufs=1) as actp,
            tc.tile_pool(name="wgp", bufs=3) as wgp,
            tc.tile_pool(name="wlp", bufs=6) as wlp,
            tc.tile_pool(name="gp", bufs=3) as gp,
            tc.tile_pool(name="orow", bufs=2) as orowp,
            tc.tile_pool(name="ps1", bufs=4, space="PSUM") as ps1,
            tc.tile_pool(name="ps2", bufs=3, space="PSUM") as ps2,
            tc.tile_pool(name="warm", bufs=1) as warmp,
            tc.tile_pool(name="warmps", bufs=1, space="PSUM") as warmpsp,
        ):
            x_sb = xp.tile([P, nchunk, KF, TBmax], dt_in)
            act_sb = actp.tile([P, KH2, C], dt_in)
            # x loads: chunk-major so chunk 0 lands first, split across the
            # scalar and gpsimd rings so the full load finishes in ~half the
            # time (whole-chunk transfers: KF*TBmax*2 contiguous B/partition).
            for c in range(nchunk):
                eng = nc.scalar if c % 2 == 0 else nc.gpsimd
                eng.dma_start(x_sb[:, c], xc[c])
            # PE warm-up: dummy matmuls bridging the ~16us until the first
            # x chunk + wg pair land, keeping the HAM clock gate at 8/8.
            wtile = warmp.tile([P, 512], mybir.dt.bfloat16)
            nc.vector.memset(wtile[:], 0.0)
            wps = warmpsp.tile([P, 512], f32)
            for _ in range(30):
                nc.tensor.matmul(wps[:], lhsT=wtile[:, :P], rhs=wtile[:],
                                 start=True, stop=True)

            for h in range(2):
                # ---- stage 1 (half h): x @ wg pairs -> act (SBUF) ----
                # Segment-outer: each segment streams its expert's wg once.
                # At kernel start, interleave the first NI pairs across chunks
                # so the PE consumes x chunks at half rate while they stream.
                units = []
                for s in range(NSEG):
                    scs = [c for c in range(nchunk) if seg_of[c] == s]
                    if h == 0 and s == 0:
                        NI = 3
                        units += [(s, i, c) for c in scs for i in range(NI)]
                        units += [(s, i, c) for i in range(NI, KH2)
                                  for c in scs]
                    else:
                        units += [(s, i, c) for i in range(KH2) for c in scs]
                pair_tiles = {}
                for (s, i, c) in units:
                    if (s, i) not in pair_tiles:
                        # One DMA per gate/lin pair: 8KB contiguous per
                        # partition -> near-peak queue rate. 3-D tile keeps
                        # the lhsT access pattern FWL-friendly.
                        wgt = wgp.tile([P, 2 * KF, P], dt_in, tag="wg")
                        gi = 2 * (KH2 * h + i)
                        nc.sync.dma_start(
                            wgt[:], wg[s][:, gi * KF:(gi + 2) * KF])
                        pair_tiles[(s, i)] = wgt
                    wgt = pair_tiles[(s, i)]
                    tb, off = sizes[c], offs[c]
                    pg = ps1.tile([P, TBmax], f32, tag="ps1")
                    pl = ps1.tile([P, TBmax], f32, tag="ps1")
                    for k in range(KF):
                        nc.tensor.matmul(
                            pg[:, :tb], lhsT=wgt[:, k, :],
                            rhs=x_sb[:, c, k, :tb],
                            start=(k == 0), stop=(k == KF - 1),
                        )
                    for k in range(KF):
                        nc.tensor.matmul(
                            pl[:, :tb], lhsT=wgt[:, KF + k, :],
                            rhs=x_sb[:, c, k, :tb],
                            start=(k == 0), stop=(k == KF - 1),
                        )
                    gtmp = gp.tile([P, TBmax], f32, tag="g")
                    nc.scalar.activation(
                        gtmp[:, :tb], pg[:, :tb],
                        mybir.ActivationFunctionType.Gelu_apprx_tanh,
                    )
                    nc.vector.tensor_mul(
                        out=act_sb[:, i, off:off + tb],
                        in0=gtmp[:, :tb], in1=pl[:, :tb],
                    )
                # ---- stage 2 (half h): act @ wl half -> y partial ----
                for m in range(MO):
                    wlts = []
                    for s in range(NSEG):
                        wlt = wlp.tile([P, KH2, P], dt_in, tag="wl")
                        nc.sync.dma_start(
                            wlt[:], wl[s][:, m, KH2 * h:KH2 * (h + 1)])
                        wlts.append(wlt)
                    o_row = orowp.tile([P, C], f32, tag="o")
                    for c in range(nchunk):
                        tb, off = sizes[c], offs[c]
                        wlt = wlts[seg_of[c]]
                        p2 = ps2.tile([P, TBmax], f32, tag="ps2")
                        for k in range(KH2):
                            nc.tensor.matmul(
                                p2[:, :tb], lhsT=wlt[:, k, :],
                                rhs=act_sb[:, k, off:off + tb],
                                start=(k == 0), stop=(k == KH2 - 1),
                            )
                        nc.vector.tensor_copy(
                            out=o_row[:, off:off + tb], in_=p2[:, :tb])
                    # Two pieces, split at the last chunk boundary: the big
                    # piece issues as soon as the second-to-last chunk's copy
                    # lands, so only the small piece trails the last matmul.
                    cs = offs[nchunk - 1]
                    nc.scalar.dma_start(yT[h][m][:, :cs], o_row[:, :cs])
                    nc.scalar.dma_start(yT[h][m][:, cs:], o_row[:, cs:])
    nc.compile()
    return nc


def _get_nc(slots, F, H, dtype_name):
    key = (tuple(slots), F, H, dtype_name)
    if key not in _NEFF_CACHE:
        _NEFF_CACHE[key] = _build_nc_fused(list(slots), F, H, dtype_name)
    return _NEFF_CACHE[key]


def _make_runner(nc, n_cores):
    """Build a sharded-jit callable for nc with outputs created in-jit (no
    per-call zero-buffer upload), so repeat launches are back-to-back."""
    import jax
    import jax.numpy as jnp
    from jax.sharding import Mesh, NamedSharding, PartitionSpec
    from jax.experimental.shard_map import shard_map
    import concourse.mybir as mybir
    from concourse.bass2jax import (
        _bass_exec_p,
        install_neuronx_cc_hook,
        partition_id_tensor,
    )

    install_neuronx_cc_hook()
    assert nc.dbg_addr is None or not nc.dbg_callbacks
    partition_name = (
        nc.partition_id_tensor.name if nc.partition_id_tensor else None
    )
    in_names, out_names, out_avals = [], [], []
    for alloc in nc.m.functions[0].allocations:
        if not isinstance(alloc, mybir.MemoryLocationSet):
            continue
        name = alloc.memorylocations[0].name
        if alloc.kind == "ExternalInput":
            if name != partition_name:
                in_names.append(name)
        elif alloc.kind == "ExternalOutput":
            out_names.append(name)
            out_avals.append(jax.core.ShapedArray(
                tuple(alloc.tensor_shape), mybir.dt.np(alloc.dtype)))
    n_params = len(in_names)
    all_names = list(in_names) + out_names
    if partition_name is not None:
        all_names.append(partition_name)

    assert nc.dbg_addr is None

    def _body(*args):
        operands = list(args)
        if partition_name is not None:
            operands.append(partition_id_tensor())
        return tuple(_bass_exec_p.bind(
            *operands,
            out_avals=tuple(out_avals),
            in_names=tuple(all_names),
            out_names=tuple(out_names),
            lowering_input_output_aliases=(),
            sim_require_finite=True,
            sim_require_nnan=True,
            nc=nc,
        ))

    devices = jax.devices()[:n_cores]
    mesh = Mesh(np.asarray(devices), ("core",))
    sharded = jax.jit(
        shard_map(
            _body, mesh=mesh,
            in_specs=(PartitionSpec("core"),) * (n_params + len(out_names)),
            out_specs=(PartitionSpec("core"),) * len(out_names),
            check_rep=False,
        ),
        keep_unused=True,
    )
    sh = NamedSharding(mesh, PartitionSpec("core"))
    return sharded, sh, in_names, out_names, out_avals


def _run_spmd_warm(nc, in_maps, n_cores, trace_cores, warm):
    """Stage inputs once, run `warm` untraced launches, then capture the NTFF
    profile of a final launch. Returns (results, perf)."""
    import glob as globmod

    import jax

    from antenv.axon_hooks import get_axon_ntff_profile_hook
    from concourse._compat import FishPath
    from concourse.bass_utils import _process_ntff_profile
    import gauge.profiler

    key = id(nc)
    if key not in _RUNNER_CACHE:
        _RUNNER_CACHE[key] = _make_runner(nc, n_cores)
    sharded, sh, in_names, out_names, out_avals = _RUNNER_CACHE[key]

    concat_in = [
        jax.device_put(
            np.concatenate(
                [np.asarray(in_maps[c][nm]) for c in range(n_cores)], axis=0
            ),
            sh,
        )
        for nm in in_names
    ]
    # Output-sized zero buffers, staged once and reused (not donated; the
    # kernel writes every output element, so reuse is safe).
    concat_in += [
        jax.device_put(
            np.zeros((n_cores * a.shape[0], *a.shape[1:]), a.dtype), sh)
        for a in out_avals
    ]
    jax.block_until_ready(concat_in)

    for _ in range(warm):
        outs = sharded(*concat_in)
        jax.block_until_ready(outs)

    hook = get_axon_ntff_profile_hook()
    tmpdir = tempfile.mkdtemp()
    if hook is None:
        outs = sharded(*concat_in)
        jax.block_until_ready(outs)
    else:
        with hook(tmpdir, list(trace_cores)):
            outs = sharded(*concat_in)
            jax.block_until_ready(outs)

    results = [
        {
            name: np.asarray(outs[i]).reshape(
                n_cores, *out_avals[i].shape)[c]
            for i, name in enumerate(out_names)
        }
        for c in range(n_cores)
    ]

    ntffs = globmod.glob(os.path.join(tmpdir, "*_body*.ntff"))
    if not ntffs:
        return results, None
    profile = gauge.profiler.Profile(
        profile_path=FishPath(tmpdir),
        kernel_dev_mode=True,
        profile_on_exit=False,
        bass_kernel=nc.m,
        offline_processing=True,
        fname="*_body*",
        metadata={"artifacts_path": tmpdir},
    )
    perf = _process_ntff_profile(
        profile, tmpdir, nc, list(range(n_cores)), list(trace_cores),
        False, {}, False,
    )
    return results, perf


def run(x, w_router, w_gating, w_linear, per_expert_scale, router_scale, top_k,
        trace=False):
    from concourse.bass_utils import BassKernelResults, run_bass_kernel_spmd

    x = np.asarray(x)
    w_router = np.asarray(w_router)
    w_gating = np.asarray(w_gating)
    w_linear = np.asarray(w_linear)
    per_expert_scale = np.asarray(per_expert_scale)
    router_scale = np.asarray(router_scale)
    k = int(top_k)

    G, S, F = x.shape
    E = w_router.shape[1]
    H = w_linear.shape[1]
    B = G * S
    assert E == 8, "expert-parallel mapping assumes 8 experts on 8 cores"
    KF, KH, MO = F // P, H // P, F // P

    choices, combine = _route(x, w_router, router_scale, k)
    wcopy = combine * per_expert_scale.astype(np.float32)[choices]

    cf = choices.reshape(-1)
    tok_of_copy = np.repeat(np.arange(B), k)
    idx_per_e = [np.nonzero(cf == e)[0] for e in range(E)]
    counts = np.array([len(ix) for ix in idx_per_e])

    slots, core_segs = _plan_segments(counts, idx_per_e, E)
    C = sum(slots)
    sizes, seg_of = _seg_chunks(slots)
    offs = np.concatenate([[0], np.cumsum(sizes)])
    nchunk = len(sizes)
    TBmax = max(sizes)
    seg_off = np.concatenate([[0], np.cumsum(slots)])

    nc = _get_nc(slots, F, H, MOE_DTYPE)
    dt_in = _np_in_dtype()

    wgts, wlts = {}, {}
    for e in range(E):
        # wg [P, MG, KF, P]: m=2i+c -> gate (c=0) / lin (c=1) rows 128i..
        wgq = w_gating[e].reshape(2, KH, P, KF, P)        # (c, i, col, ko, p)
        wgts[e] = np.ascontiguousarray(
            wgq.transpose(4, 1, 0, 3, 2).reshape(P, 2 * KH * KF, P)
        ).astype(dt_in)
        # wl [P, MO, KH, P]: wl[p, m, kh, col] = w_linear[e][kh*P+p, m*P+col]
        wlq = w_linear[e].reshape(KH, P, MO, P)           # (kh, p, m, col)
        wlts[e] = np.ascontiguousarray(
            wlq.transpose(1, 2, 0, 3)).astype(dt_in)

    xf = x.reshape(B, F)
    in_maps = []
    for segs in core_segs:
        # xc [nchunk, P, KF, TBmax]: xc[c, p, kf, t] = x[tok(c, t), kf*P+p]
        xf_pad = np.zeros((nchunk * TBmax, F), dtype=dt_in)
        for c in range(nchunk):
            s = seg_of[c]
            lo = offs[c] - seg_off[s]
            _, copy_idx = segs[s]
            ci = copy_idx[lo:lo + sizes[c]]
            seg = xf[tok_of_copy[ci]].astype(dt_in)
            xf_pad[c * TBmax:c * TBmax + len(seg)] = seg
        xcq = np.ascontiguousarray(
            xf_pad.reshape(nchunk, TBmax, KF, P).transpose(0, 3, 2, 1)
        )
        wgq = np.stack([wgts[e] for e, _ in segs])
        wlq = np.stack([wlts[e] for e, _ in segs])
        in_maps.append({"xc": xcq, "wg": wgq, "wl": wlq})

    if trace:
        results, perf = _run_spmd_warm(
            nc, in_maps, E, list(range(E)), MOE_WARM)
        if perf is not None:
            res = BassKernelResults(
                results=results,
                instructions_and_trace=perf.insts_and_trace_path,
                profile_json=perf.profile_json,
                exec_time_ns=perf.exec_time_ns,
                mean_exec_time_ns=perf.mean_exec_time_ns,
                max_exec_time_core_id=perf.max_exec_time_core_id,
                per_core_scope_times=perf.per_core_scope_times,
            )
        else:
            res = BassKernelResults(
                results=results, instructions_and_trace=None,
                profile_json=None, exec_time_ns=None)
    else:
        res = run_bass_kernel_spmd(nc, in_maps, core_ids=list(range(E)))

    out = np.zeros((B, F), dtype=np.float32)
    wflat = wcopy.reshape(-1)
    for core, segs in enumerate(core_segs):
        yT = res.results[core]["yT"]                      # [2, MO, P, C]
        yT = yT.sum(axis=0, dtype=np.float32)
        y = yT.transpose(2, 0, 1).reshape(C, F)           # column-major tokens
        for s, (e, copy_idx) in enumerate(segs):
            n_s = len(copy_idx)
            if n_s == 0:
                continue
            ys = y[seg_off[s]:seg_off[s] + n_s]
            toks = tok_of_copy[copy_idx]
            out[toks] += wflat[copy_idx][:, None] * ys
    return out.reshape(G, S, F), res


def kernel(**inputs) -> np.ndarray:
    out, _ = run(**inputs)
    return out


# revision 5
# speedup vs baseline: 1.0574x; 1.0574x over previous
"""MoE ragged FFN kernel for Trainium2 (8 NeuronCores, pair-balanced).

v4 = v3 (fused, act SBUF-resident, H halves) + load balancing: experts are
paired (largest count with smallest), each pair's tokens split across two
cores. Every core runs two segments — slot1 tokens of expert A, slot2 of
expert B — so capacity drops from max(count) to
even_ceil(max_big/2) + even_ceil(max_small/2) (~mean), cutting padded
compute on every core. Each core streams both experts' weights (wg/wl get a
leading [2] segment dim).

The traced path stages inputs on device once, runs `warm` untraced launches
back-to-back, then captures the NTFF profile of a final launch — measuring
steady-state (power-controller ramped) rather than cold-start behavior.
"""

import os
import tempfile

import numpy as np

P = 128
RMS_EPS = 1e-6

MOE_DTYPE = os.environ.get("MOE_DTYPE", "f16")
MOE_WARM = int(os.environ.get("MOE_WARM", "1"))

_NEFF_CACHE: dict = {}
_RUNNER_CACHE: dict = {}


def _route_numpy(x, w_router, router_scale, top_k):
    G, S, F = x.shape
    B = G * S
    var = np.mean(np.square(x), axis=-1, keepdims=True, dtype=np.float32)
    ri = x / np.sqrt(var + RMS_EPS)
    ri = ri * np.float32(1.0 / np.sqrt(np.float32(F))) * router_scale
    logits = (ri.reshape(B, F) @ w_router).astype(np.float32)
    m = logits.max(axis=-1, keepdims=True)
    e = np.exp(logits - m)
    probs = e / e.sum(axis=-1, keepdims=True)
    choices = np.argsort(-logits, axis=-1, kind="stable")[:, :top_k]
    sel = np.take_along_axis(probs, choices, axis=-1)
    renorm = sel.sum(axis=-1, keepdims=True)
    renorm = np.where(renorm > 0.0, renorm, np.float32(1.0))
    combine = (sel / renorm).astype(np.float32)
    return choices.astype(np.int64), combine


def _route(x, w_router, router_scale, top_k):
    """Reference-exact router on CPU via jax. Returns (choices, combine)."""
    try:
        import jax
        import jax.numpy as jnp

        cpu = jax.devices("cpu")[0]
    except Exception:
        return _route_numpy(np.asarray(x, dtype=np.float32),
                            np.asarray(w_router), np.asarray(router_scale),
                            top_k)
    G, S, F = x.shape
    E = w_router.shape[1]
    with jax.default_device(cpu):
        xj = jax.device_put(np.asarray(x), cpu)
        wj = jax.device_put(np.asarray(w_router), cpu)
        rj = jax.device_put(np.asarray(router_scale), cpu)
        var = jnp.mean(jnp.square(xj), axis=-1, keepdims=True)
        ri = xj * jax.lax.rsqrt(var + RMS_EPS)
        root_size = jax.lax.rsqrt(jnp.array(F, dtype=ri.dtype))
        ri = ri * root_size * rj.astype(ri.dtype)
        logits = jnp.einsum("gsd,de->gse", ri, wj).astype(jnp.float32)
        probs = jax.nn.softmax(logits, axis=-1)
        _, choices = jax.lax.approx_max_k(logits, k=top_k)
        indicator = jax.nn.one_hot(choices, E, dtype=probs.dtype).sum(axis=-2)
        renorm = jnp.sum(indicator * probs, axis=-1, keepdims=True)
        renorm = jnp.where(renorm > 0.0, renorm, 1.0)
        weights = probs / renorm
        combine = jnp.take_along_axis(weights, choices, axis=-1)
    B = G * S
    return (
        np.asarray(choices).reshape(B, top_k),
        np.asarray(combine).reshape(B, top_k).astype(np.float32),
    )


def _mm_dt(mybir, dtype_name):
    return {
        "f32r": mybir.dt.float32r,
        "bf16": mybir.dt.bfloat16,
        "f16": mybir.dt.float16,
    }[dtype_name]


def _np_in_dtype():
    if MOE_DTYPE == "f32r":
        return np.float32
    if MOE_DTYPE == "f16":
        return np.float16
    import ml_dtypes

    return ml_dtypes.bfloat16


def _chunk_sizes(C, cap=512):
    """Near-equal even chunk sizes <= cap summing to C."""
    assert C % 2 == 0
    nchunk = -(-C // cap)
    base = 2 * (C // (2 * nchunk))
    rem = (C - nchunk * base) // 2
    return [base + 2 * (c < rem) for c in range(nchunk)]


def _plan_segments(counts, idx_per_e, E):
    """Choose uniform per-core slot sizes and the expert->slot-instance
    assignment. Tries a 3-segment tercile template (top-2 experts spread over
    3 cores each), falls back to big/small pairing, then to one expert/core.

    Returns (slots, core_segs) where core_segs[core] is a list of
    (expert, token_copy_indices) per segment."""
    def ec(v, n):                     # even ceil of v/n
        return max(2, 2 * (-(-int(v) // (2 * n))))

    order = [int(e) for e in np.argsort(-counts, kind="stable")]
    c = [int(counts[e]) for e in order]

    plans = []
    if E == 8:
        # --- 3-seg template ---
        s1 = max(ec(c[0], 3), ec(c[1], 3))
        s2 = max(ec(max(c[2] - s1, 0), 2), ec(max(c[3] - s1, 0), 2))
        s3 = max(ec(max(c[i] - s2, 0), 2) for i in range(4, 8))
        # slot instances per core, in slot order; entries are expert-rank
        a1 = [0, 0, 0, 1, 1, 1, 2, 3]
        a2 = [2, 2, 3, 3, 4, 5, 6, 7]
        a3 = [4, 4, 5, 5, 6, 6, 7, 7]
        if min(s1, s2, s3) >= 300:
            plans.append(([s1, s2, s3], [a1, a2, a3]))
        # --- 2-seg pairing ---
        p1 = max(ec(c[i], 2) for i in range(4))
        p2 = max(ec(c[i], 2) for i in range(4, 8))
        a1 = [0, 0, 1, 1, 2, 2, 3, 3]
        a2 = [7, 7, 6, 6, 5, 5, 4, 4]
        if min(p1, p2) >= 300:
            plans.append(([p1, p2], [a1, a2]))
    # --- 1-seg fallback: one expert per core ---
    if E == 8:
        s = max(512, 2 * (-(-int(counts.max()) // 2)))
        plans.append(([s], [list(range(8))]))

    slots, assign = min(plans, key=lambda p: sum(p[0]))

    # Split each expert's token list sequentially over its slot instances
    # (core-major order within each slot row, slot rows in order).
    core_segs = [[] for _ in range(E)]
    taken = {e: 0 for e in range(E)}
    for s, row in enumerate(assign):
        for core in range(E):
            e = order[row[core]]
            lo = taken[e]
            part = idx_per_e[e][lo:lo + slots[s]]
            taken[e] = lo + len(part)
            core_segs[core].append((e, part))
    for e in range(E):
        assert taken[e] == len(idx_per_e[e]), (
            f"segment plan dropped tokens for expert {e}")
    return slots, core_segs


def _seg_chunks(slots):
    """Per-segment chunk lists. The 432 cap keeps chunks near-equal ACROSS
    segments too, so the x tile (padded to the global max chunk) stays small
    enough for SBUF."""
    sizes, seg_of = [], []
    for s, slot in enumerate(slots):
        cs = _chunk_sizes(slot, cap=432)
        sizes += cs
        seg_of += [s] * len(cs)
    return sizes, seg_of


def _build_nc_fused(slots, F, H, dtype_name):
    import concourse.mybir as mybir
    import concourse.tile as tile
    from concourse import bacc

    KF = F // P          # stage-1 contraction subtiles
    KH = H // P
    KH2 = KH // 2        # stage-2 contraction subtiles per half
    MG = 2 * H // P      # wg column tiles, gate/lin interleaved per 128
    MO = F // P          # output row tiles
    C = sum(slots)
    NSEG = len(slots)
    sizes, seg_of = _seg_chunks(slots)
    offs = np.concatenate([[0], np.cumsum(sizes)]).tolist()
    nchunk = len(sizes)
    TBmax = max(sizes)
    f32 = mybir.dt.float32
    dt_in = _mm_dt(mybir, dtype_name)

    nc = bacc.Bacc(None, target_bir_lowering=False)
    xc = nc.dram_tensor("xc", [nchunk, P, KF, TBmax], dt_in,
                        kind="ExternalInput")
    wg = nc.dram_tensor("wg", [NSEG, P, MG * KF, P], dt_in,
                        kind="ExternalInput")
    wl = nc.dram_tensor("wl", [NSEG, P, MO, KH, P], dt_in,
                        kind="ExternalInput")
    yT = nc.dram_tensor("yT", [2, MO, P, C], f32, kind="ExternalOutput")

    with tile.TileContext(nc) as tc:
        with (
            tc.tile_pool(name="xp", bufs=1) as xp,
            tc.tile_pool(name="actp", bufs=1) as actp,
            tc.tile_pool(name="wgp", bufs=3) as wgp,
            tc.tile_pool(name="wlp", bufs=6) as wlp,
            tc.tile_pool(name="gp", bufs=3) as gp,
            tc.tile_pool(name="orow", bufs=2) as orowp,
            tc.tile_pool(name="ps1", bufs=4, space="PSUM") as ps1,
            tc.tile_pool(name="ps2", bufs=3, space="PSUM") as ps2,
            tc.tile_pool(name="warm", bufs=1) as warmp,
            tc.tile_pool(name="warmps", bufs=1, space="PSUM") as warmpsp,
        ):
            x_sb = xp.tile([P, nchunk, KF, TBmax], dt_in)
            act_sb = actp.tile([P, KH2, C], dt_in)
            # x loads: chunk-major so chunk 0 lands first; scalar HWDGE ring
            # only (the gpsimd SWDGE ring adds ~10us of program-load latency
            # that delays early chunks). Whole-chunk transfers:
            # KF*TBmax*2 contiguous bytes per partition.
            for c in range(nchunk):
                nc.scalar.dma_start(x_sb[:, c], xc[c])
            # PE warm-up: dummy matmuls bridging the ~25us until the first
            # x chunk + wg pair land, keeping the HAM clock gate at 8/8.
            wtile = warmp.tile([P, 512], mybir.dt.bfloat16)
            nc.vector.memset(wtile[:], 0.0)
            wps = warmpsp.tile([P, 512], f32)
            for _ in range(40):
                nc.tensor.matmul(wps[:], lhsT=wtile[:, :P], rhs=wtile[:],
                                 start=True, stop=True)

            for h in range(2):
                # ---- stage 1 (half h): x @ wg pairs -> act (SBUF) ----
                # Segment-outer: each segment streams its expert's wg once.
                # At kernel start, interleave the first NI pairs across chunks
                # so the PE consumes x chunks at half rate while they stream.
                units = []
                for s in range(NSEG):
                    scs = [c for c in range(nchunk) if seg_of[c] == s]
                    if h == 0 and s == 0:
                        NI = 3
                        units += [(s, i, c) for c in scs for i in range(NI)]
                        units += [(s, i, c) for i in range(NI, KH2)
                                  for c in scs]
                    else:
                        units += [(s, i, c) for i in range(KH2) for c in scs]
                pair_tiles = {}
                for (s, i, c) in units:
                    if (s, i) not in pair_tiles:
                        # One DMA per gate/lin pair: 8KB contiguous per
                        # partition -> near-peak queue rate. 3-D tile keeps
                        # the lhsT access pattern FWL-friendly.
                        wgt = wgp.tile([P, 2 * KF, P], dt_in, tag="wg")
                        gi = 2 * (KH2 * h + i)
                        nc.sync.dma_start(
                            wgt[:], wg[s][:, gi * KF:(gi + 2) * KF])
                        pair_tiles[(s, i)] = wgt
                    wgt = pair_tiles[(s, i)]
                    tb, off = sizes[c], offs[c]
                    pg = ps1.tile([P, TBmax], f32, tag="ps1")
                    pl = ps1.tile([P, TBmax], f32, tag="ps1")
                    for k in range(KF):
                        nc.tensor.matmul(
                            pg[:, :tb], lhsT=wgt[:, k, :],
                            rhs=x_sb[:, c, k, :tb],
                            start=(k == 0), stop=(k == KF - 1),
                        )
                    for k in range(KF):
                        nc.tensor.matmul(
                            pl[:, :tb], lhsT=wgt[:, KF + k, :],
                            rhs=x_sb[:, c, k, :tb],
                            start=(k == 0), stop=(k == KF - 1),
                        )
                    gtmp = gp.tile([P, TBmax], f32, tag="g")
                    nc.scalar.activation(
                        gtmp[:, :tb], pg[:, :tb],
                        mybir.ActivationFunctionType.Gelu_apprx_tanh,
                    )
                    nc.vector.tensor_mul(
                        out=act_sb[:, i, off:off + tb],
                        in0=gtmp[:, :tb], in1=pl[:, :tb],
                    )
                # ---- stage 2 (half h): act @ wl half -> y partial ----
                for m in range(MO):
                    wlts = []
                    for s in range(NSEG):
                        wlt = wlp.tile([P, KH2, P], dt_in, tag="wl")
                        nc.sync.dma_start(
                            wlt[:], wl[s][:, m, KH2 * h:KH2 * (h + 1)])
                        wlts.append(wlt)
                    o_row = orowp.tile([P, C], f32, tag="o")
                    for c in range(nchunk):
                        tb, off = sizes[c], offs[c]
                        wlt = wlts[seg_of[c]]
                        p2 = ps2.tile([P, TBmax], f32, tag="ps2")
                        for k in range(KH2):
                            nc.tensor.matmul(
                                p2[:, :tb], lhsT=wlt[:, k, :],
                                rhs=act_sb[:, k, off:off + tb],
                                start=(k == 0), stop=(k == KH2 - 1),
                            )
                        nc.vector.tensor_copy(
                            out=o_row[:, off:off + tb], in_=p2[:, :tb])
                    # Two pieces, split at the last chunk boundary: the big
                    # piece issues as soon as the second-to-last chunk's copy
                    # lands, so only the small piece trails the last matmul.
                    cs = offs[nchunk - 1]
                    nc.scalar.dma_start(yT[h][m][:, :cs], o_row[:, :cs])
                    nc.scalar.dma_start(yT[h][m][:, cs:], o_row[:, cs:])
    nc.compile()
    return nc


def _get_nc(slots, F, H, dtype_name):
    key = (tuple(slots), F, H, dtype_name)
    if key not in _NEFF_CACHE:
        _NEFF_CACHE[key] = _build_nc_fused(list(slots), F, H, dtype_name)
    return _NEFF_CACHE[key]


def _make_runner(nc, n_cores):
    """Build a sharded-jit callable for nc with outputs created in-jit (no
    per-call zero-buffer upload), so repeat launches are back-to-back."""
    import jax
    import jax.numpy as jnp
    from jax.sharding import Mesh, NamedSharding, PartitionSpec
    from jax.experimental.shard_map import shard_map
    import concourse.mybir as mybir
    from concourse.bass2jax import (
        _bass_exec_p,
        install_neuronx_cc_hook,
        partition_id_tensor,
    )

    install_neuronx_cc_hook()
    assert nc.dbg_addr is None or not nc.dbg_callbacks
    partition_name = (
        nc.partition_id_tensor.name if nc.partition_id_tensor else None
    )
    in_names, out_names, out_avals = [], [], []
    for alloc in nc.m.functions[0].allocations:
        if not isinstance(alloc, mybir.MemoryLocationSet):
            continue
        name = alloc.memorylocations[0].name
        if alloc.kind == "ExternalInput":
            if name != partition_name:
                in_names.append(name)
        elif alloc.kind == "ExternalOutput":
            out_names.append(name)
            out_avals.append(jax.core.ShapedArray(
                tuple(alloc.tensor_shape), mybir.dt.np(alloc.dtype)))
    n_params = len(in_names)
    all_names = list(in_names) + out_names
    if partition_name is not None:
        all_names.append(partition_name)

    assert nc.dbg_addr is None

    def _body(*args):
        operands = list(args)
        if partition_name is not None:
            operands.append(partition_id_tensor())
        return tuple(_bass_exec_p.bind(
            *operands,
            out_avals=tuple(out_avals),
            in_names=tuple(all_names),
            out_names=tuple(out_names),
            lowering_input_output_aliases=(),
            sim_require_finite=True,
            sim_require_nnan=True,
            nc=nc,
        ))

    devices = jax.devices()[:n_cores]
    mesh = Mesh(np.asarray(devices), ("core",))
    sharded = jax.jit(
        shard_map(
            _body, mesh=mesh,
            in_specs=(PartitionSpec("core"),) * (n_params + len(out_names)),
            out_specs=(PartitionSpec("core"),) * len(out_names),
            check_rep=False,
        ),
        keep_unused=True,
    )
    sh = NamedSharding(mesh, PartitionSpec("core"))
    return sharded, sh, in_names, out_names, out_avals


def _run_spmd_warm(nc, in_maps, n_cores, trace_cores, warm):
    """Stage inputs once, run `warm` untraced launches, then capture the NTFF
    profile of a final launch. Returns (results, perf)."""
    import glob as globmod

    import jax

    from antenv.axon_hooks import get_axon_ntff_profile_hook
    from concourse._compat import FishPath
    from concourse.bass_utils import _process_ntff_profile
    import gauge.profiler

    key = id(nc)
    if key not in _RUNNER_CACHE:
        _RUNNER_CACHE[key] = _make_runner(nc, n_cores)
    sharded, sh, in_names, out_names, out_avals = _RUNNER_CACHE[key]

    concat_in = [
        jax.device_put(
            np.concatenate(
                [np.asarray(in_maps[c][nm]) for c in range(n_cores)], axis=0
            ),
            sh,
        )
        for nm in in_names
    ]
    # Output-sized zero buffers, staged once and reused (not donated; the
    # kernel writes every output element, so reuse is safe).
    concat_in += [
        jax.device_put(
            np.zeros((n_cores * a.shape[0], *a.shape[1:]), a.dtype), sh)
        for a in out_avals
    ]
    jax.block_until_ready(concat_in)

    for _ in range(warm):
        outs = sharded(*concat_in)
        jax.block_until_ready(outs)

    hook = get_axon_ntff_profile_hook()
    tmpdir = tempfile.mkdtemp()
    if hook is None:
        outs = sharded(*concat_in)
        jax.block_until_ready(outs)
    else:
        with hook(tmpdir, list(trace_cores)):
            outs = sharded(*concat_in)
            jax.block_until_ready(outs)

    results = [
        {
            name: np.asarray(outs[i]).reshape(
                n_cores, *out_avals[i].shape)[c]
            for i, name in enumerate(out_names)
        }
        for c in range(n_cores)
    ]

    ntffs = globmod.glob(os.path.join(tmpdir, "*_body*.ntff"))
    if not ntffs:
        return results, None
    profile = gauge.profiler.Profile(
        profile_path=FishPath(tmpdir),
        kernel_dev_mode=True,
        profile_on_exit=False,
        bass_kernel=nc.m,
        offline_processing=True,
        fname="*_body*",
        metadata={"artifacts_path": tmpdir},
    )
    perf = _process_ntff_profile(
        profile, tmpdir, nc, list(range(n_cores)), list(trace_cores),
        False, {}, False,
    )
    return results, perf


def run(x, w_router, w_gating, w_linear, per_expert_scale, router_scale, top_k,
        trace=False):
    from concourse.bass_utils import BassKernelResults, run_bass_kernel_spmd

    x = np.asarray(x)
    w_router = np.asarray(w_router)
    w_gating = np.asarray(w_gating)
    w_linear = np.asarray(w_linear)
    per_expert_scale = np.asarray(per_expert_scale)
    router_scale = np.asarray(router_scale)
    k = int(top_k)

    G, S, F = x.shape
    E = w_router.shape[1]
    H = w_linear.shape[1]
    B = G * S
    assert E == 8, "expert-parallel mapping assumes 8 experts on 8 cores"
    KF, KH, MO = F // P, H // P, F // P

    choices, combine = _route(x, w_router, router_scale, k)
    wcopy = combine * per_expert_scale.astype(np.float32)[choices]

    cf = choices.reshape(-1)
    tok_of_copy = np.repeat(np.arange(B), k)
    idx_per_e = [np.nonzero(cf == e)[0] for e in range(E)]
    counts = np.array([len(ix) for ix in idx_per_e])

    slots, core_segs = _plan_segments(counts, idx_per_e, E)
    C = sum(slots)
    sizes, seg_of = _seg_chunks(slots)
    offs = np.concatenate([[0], np.cumsum(sizes)])
    nchunk = len(sizes)
    TBmax = max(sizes)
    seg_off = np.concatenate([[0], np.cumsum(slots)])

    nc = _get_nc(slots, F, H, MOE_DTYPE)
    dt_in = _np_in_dtype()

    wgts, wlts = {}, {}
    for e in range(E):
        # wg [P, MG, KF, P]: m=2i+c -> gate (c=0) / lin (c=1) rows 128i..
        wgq = w_gating[e].reshape(2, KH, P, KF, P)        # (c, i, col, ko, p)
        wgts[e] = np.ascontiguousarray(
            wgq.transpose(4, 1, 0, 3, 2).reshape(P, 2 * KH * KF, P)
        ).astype(dt_in)
        # wl [P, MO, KH, P]: wl[p, m, kh, col] = w_linear[e][kh*P+p, m*P+col]
        wlq = w_linear[e].reshape(KH, P, MO, P)           # (kh, p, m, col)
        wlts[e] = np.ascontiguousarray(
            wlq.transpose(1, 2, 0, 3)).astype(dt_in)

    xf = x.reshape(B, F)
    in_maps = []
    for segs in core_segs:
        # xc [nchunk, P, KF, TBmax]: xc[c, p, kf, t] = x[tok(c, t), kf*P+p]
        xf_pad = np.zeros((nchunk * TBmax, F), dtype=dt_in)
        for c in range(nchunk):
            s = seg_of[c]
            lo = offs[c] - seg_off[s]
            _, copy_idx = segs[s]
            ci = copy_idx[lo:lo + sizes[c]]
            seg = xf[tok_of_copy[ci]].astype(dt_in)
            xf_pad[c * TBmax:c * TBmax + len(seg)] = seg
        xcq = np.ascontiguousarray(
            xf_pad.reshape(nchunk, TBmax, KF, P).transpose(0, 3, 2, 1)
        )
        wgq = np.stack([wgts[e] for e, _ in segs])
        wlq = np.stack([wlts[e] for e, _ in segs])
        in_maps.append({"xc": xcq, "wg": wgq, "wl": wlq})

    if trace:
        tc_env = os.environ.get("MOE_TRACE_CORES")
        tcores = ([int(t) for t in tc_env.split(",")] if tc_env
                  else list(range(E)))
        results, perf = _run_spmd_warm(
            nc, in_maps, E, tcores, MOE_WARM)
        if perf is not None:
            res = BassKernelResults(
                results=results,
                instructions_and_trace=perf.insts_and_trace_path,
                profile_json=perf.profile_json,
                exec_time_ns=perf.exec_time_ns,
                mean_exec_time_ns=perf.mean_exec_time_ns,
                max_exec_time_core_id=perf.max_exec_time_core_id,
                per_core_scope_times=perf.per_core_scope_times,
            )
        else:
            res = BassKernelResults(
                results=results, instructions_and_trace=None,
                profile_json=None, exec_time_ns=None)
    else:
        res = run_bass_kernel_spmd(nc, in_maps, core_ids=list(range(E)))

    out = np.zeros((B, F), dtype=np.float32)
    wflat = wcopy.reshape(-1)
    for core, segs in enumerate(core_segs):
        yT = res.results[core]["yT"]                      # [2, MO, P, C]
        yT = yT.sum(axis=0, dtype=np.float32)
        y = yT.transpose(2, 0, 1).reshape(C, F)           # column-major tokens
        for s, (e, copy_idx) in enumerate(segs):
            n_s = len(copy_idx)
            if n_s == 0:
                continue
            ys = y[seg_off[s]:seg_off[s] + n_s]
            toks = tok_of_copy[copy_idx]
            out[toks] += wflat[copy_idx][:, None] * ys
    return out.reshape(G, S, F), res


def kernel(**inputs) -> np.ndarray:
    out, _ = run(**inputs)
    return out
